# revision 26
# baseline (speedup 1.0000x reference)
"""Trainium2 Bass kernel for the 4-modality attention-fusion module.

Computes, for full inputs mod0..mod3 [16384, 1024] f32 and W [1024, 1024] f32:
    scores_m = mod_m @ W.T                      (per modality)
    attn     = softmax over m of scores         (elementwise over [B, L])
    fused    = sum_m mod_m * attn_m
    scaler_b = 1 + #{m : sum_l mod_m[b, l] == 0}
    out      = fused * scaler[:, None]

Sharded data-parallel over 8 NeuronCores along the batch dim (2048 rows each),
W replicated. Design (PE runs ONLY the score matmuls):
  - W.T is built resident in SBUF in bf16 once, via XBAR DMA transposes
    (wt[p, j, k] = W[k, j*128 + p]);
  - per 128-patient tile, the f32 mod tiles are cast to bf16 on ACT; that
    same ACT op carries accum_out, producing the per-row sums (zero-modality
    detection) for free;
  - the bf16 mod tiles are transposed by the DMA XBAR engine (SBUF->SBUF,
    ~0.9us per [128,1024]) into the matmul stationary layout — the PE does
    no transposes and the ACT no PSUM evictions;
  - bf16 matmuls accumulate scores in PSUM (1 col/cycle like f32r, but with
    fast weight load and no PE transpose overhead);
  - softmax over the 4 modalities entirely on DVE: exp on ACT straight out
    of PSUM to bf16 tiles, numerator products/sums in bf16 (2x DVE mode),
    denominator accumulated to f32 for the fast-reciprocal op, zero-modality
    rescale folded into the final scalar_tensor_tensor.  Pool (gpsimd) is
    deliberately idle: its Add/Multiply run far below roofline on HW and
    became the critical path when given the denominator chain;
  - per-segment emission order keeps next-tile casts ahead of this tile's
    exps in the ACT queue and next-tile loads ahead of the XBAR transposes
    in the SP queue; modt/psum rings are deep enough (3 and 8 banks) that
    the PE never waits on stationaries or exp drains.

Measured on 8 axon trn2 cores via the R-repeat slope method: ~350 us per
iteration (baseline f32r/PE-transpose kernel: ~437 us by the harness's
measurement, ~358 us by this harness's method), max rel err ~7.5e-3.
"""

import sys

sys.path.insert(0, "/opt/trn_rl_repo")

from contextlib import ExitStack

import numpy as np

import concourse.bass as bass
import concourse.bacc as bacc
import concourse.mybir as mybir
import concourse.tile as tile
from concourse.bass_utils import run_bass_kernel_spmd

F32 = mybir.dt.float32
BF16 = mybir.dt.bfloat16
AF = mybir.ActivationFunctionType

N_CORES = 8
B_FULL = 16384
L = 1024
P = 128
B_SHARD = B_FULL // N_CORES          # 2048
NPT = B_SHARD // P                   # 16 patient tiles per core
NM = 4                               # modalities
NLC = L // P                         # 8 l-chunks (contraction)
NH = 2                               # k halves
KH = L // NH                         # 512

_CACHE: dict = {}


def _build(
    repeat: int = 1,
    *,
    elem: bool = True,
    transp: bool = True,
    tail_bf16: bool = True,
):
    E_DT = BF16 if tail_bf16 else F32
    nc = bacc.Bacc("TRN2", target_bir_lowering=False, debug=False)
    mods_d = [
        nc.dram_tensor(f"mod{m}", [B_SHARD, L], F32, kind="ExternalInput").ap()
        for m in range(NM)
    ]
    w_d = nc.dram_tensor("W", [L, L], F32, kind="ExternalInput").ap()
    out_d = nc.dram_tensor("out", [B_SHARD, L], F32, kind="ExternalOutput").ap()

    with tile.TileContext(nc) as tc, ExitStack() as ctx:
        wt_p = ctx.enter_context(tc.tile_pool(name="wt", bufs=1))
        wload_p = ctx.enter_context(tc.tile_pool(name="wload", bufs=2))
        mod_p = ctx.enter_context(
            tc.tile_pool(name="mod", bufs=3 if tail_bf16 else 4)
        )
        modb_p = ctx.enter_context(
            tc.tile_pool(name="modb", bufs=4 if tail_bf16 else 2)
        )
        modt_p = ctx.enter_context(tc.tile_pool(name="modt", bufs=3))
        e_p = ctx.enter_context(tc.tile_pool(name="e", bufs=6))
        rs_p = ctx.enter_context(tc.tile_pool(name="rs", bufs=4))
        tmp_p = ctx.enter_context(tc.tile_pool(name="tmp", bufs=2))
        out_p = ctx.enter_context(tc.tile_pool(name="outp", bufs=2))
        ps_q = ctx.enter_context(
            tc.tile_pool(name="ps_q", bufs=8, space=bass.MemorySpace.PSUM)
        )

        # ---- WT resident in SBUF (bf16): wt[p, j, k] = W[k, j*128 + p] ----
        wt = wt_p.tile([P, NLC, L], BF16, tag="wt")
        for kc in range(NLC):
            wk = wload_p.tile([P, L], F32, tag="wk")
            nc.sync.dma_start(wk[:], w_d[kc * P : (kc + 1) * P, :])
            wkb = wload_p.tile([P, L], BF16, tag="wkb")
            nc.scalar.copy(wkb[:], wk[:])
            nc.sync.dma_start_transpose(wt[:, :, kc * P : (kc + 1) * P], wkb[:])

        # ---------------- main loop, software-pipelined ----------------
        rep_cm = (
            tc.For_i(
                0,
                repeat,
                1,
                hint_engines=(
                    mybir.EngineType.PE,
                    mybir.EngineType.DVE,
                    mybir.EngineType.Activation,
                    mybir.EngineType.Pool,
                    mybir.EngineType.SP,
                ),
            )
            if repeat > 1
            else None
        )
        if rep_cm is not None:
            rep_cm.__enter__()

        def emit_load(p):
            row = slice(p * P, (p + 1) * P)
            mods = []
            for m in range(NM):
                mt = mod_p.tile([P, L], F32, tag=f"mod{m}")
                nc.sync.dma_start(mt[:], mods_d[m][row, :])
                mods.append(mt)
            return mods

        def emit_conv(p, mods):
            """f32 -> bf16 casts on ACT into ONE fused [P, 4*L] tile;
            accum_out rides along to produce the per-modality row sums
            (zero-modality detection) for free."""
            rsum = rs_p.tile([P, NM], F32, tag="rsum")
            modb = modb_p.tile([P, NM * L], BF16, tag="modb")
            for m in range(NM):
                nc.scalar.activation(
                    modb[:, m * L : (m + 1) * L],
                    mods[m][:],
                    AF.Copy,
                    accum_out=rsum[:, m : m + 1],
                )
            return modb, rsum

        def emit_transp(p, modb):
            """ONE XBAR transpose per patient tile (not 4): the fused
            [P, 4096] -> [P, 32, 128] transfer amortizes the per-instruction
            DMA-drain overhead the transpose deadlock-guard imposes."""
            if not transp:
                return None
            mT = modt_p.tile([P, NM * NLC, P], BF16, tag="modt")
            nc.sync.dma_start_transpose(mT[:], modb[:])
            return mT

        def emit_pe(p, modts):
            """Score matmuls + trailing exps for tile p.

            Modalities run in interleaved PAIRS: within each j, consecutive
            matmuls alternate between the two modalities' stationaries, so
            each LDWEIGHTS can target the PE's background weight buffer and
            load during the previous matmul's stream (the reorder window
            only pulls LDWEIGHTS ahead for a non-conflicting buffer).  The
            four open accumulation chains per pair plus the previous pair's
            draining banks exactly fill the 8 PSUM banks."""
            es = {}
            for pair in ((0, 1), (2, 3)):
                sqs = {}
                for m in pair:
                    for h in range(NH):
                        sq = ps_q.tile([P, KH], F32, tag="sq")
                        sqs[(m, h)] = sq
                for j in range(NLC):
                    for h in range(NH):
                        for m in pair:
                            lhsT = (
                                modts[:, m * NLC + j, :]
                                if modts is not None
                                else wt[:, j, 0:P]
                            )
                            nc.tensor.matmul(
                                sqs[(m, h)][:],
                                lhsT,
                                wt[:, j, h * KH : (h + 1) * KH],
                                start=(j == 0),
                                stop=(j == NLC - 1),
                            )
                if elem:
                    for m in pair:
                        for h in range(NH):
                            e = e_p.tile([P, KH], E_DT, tag=f"e{m}")
                            nc.scalar.activation(e[:], sqs[(m, h)][:], AF.Exp)
                            es[(m, h)] = e
            return es

        def emit_tail(state):
            """Softmax combine + output for tile p (lags one segment)."""
            p, mods, es, rsum = state
            row = slice(p * P, (p + 1) * P)
            zt = tmp_p.tile([P, NM], F32, tag="zt")
            zs = tmp_p.tile([P, 1], F32, tag="zs")
            nc.vector.tensor_scalar(
                out=zt[:],
                in0=rsum[:],
                scalar1=0.0,
                scalar2=None,
                op0=mybir.AluOpType.is_equal,
                op1=mybir.AluOpType.add,
                accum_out=zs[:],
            )
            scaler = tmp_p.tile([P, 1], F32, tag="scaler")
            nc.vector.tensor_scalar_add(scaler[:], zs[:], 1.0)
            ot = out_p.tile([P, L], F32, tag="ot")
            for h in range(NH):
                e0, e1, e2, e3 = (es[(m, h)] for m in range(NM))
                # tail tensor ops mostly on DVE: gpsimd (Pool) Add/Multiply
                # run far below roofline on HW (software Q7 implementation)
                # and became the critical path when given the whole
                # denominator chain.  Pool gets exactly ONE add per half
                # (d23), hidden behind the first two DVE muls.
                def msrc(m):
                    if tail_bf16:
                        return mods[:, m * L + h * KH : m * L + (h + 1) * KH]
                    return mods[m][:, h * KH : (h + 1) * KH]

                d01 = tmp_p.tile([P, KH], F32, tag="d01")
                d23 = tmp_p.tile([P, KH], F32, tag="d23")
                nc.vector.tensor_add(d01[:], e0[:], e1[:])
                nc.gpsimd.tensor_add(d23[:], e2[:], e3[:])
                nc.vector.tensor_mul(e0[:], e0[:], msrc(0))
                nc.vector.tensor_mul(e1[:], e1[:], msrc(1))
                nc.vector.tensor_add(d01[:], d01[:], d23[:])
                nc.vector.tensor_mul(e2[:], e2[:], msrc(2))
                nc.vector.tensor_mul(e3[:], e3[:], msrc(3))
                nc.vector.tensor_add(e0[:], e0[:], e1[:])
                nc.vector.tensor_add(e2[:], e2[:], e3[:])
                nc.vector.reciprocal_approx_fast(out=d01[:], in_=d01[:])
                nc.vector.tensor_add(e0[:], e0[:], e2[:])
                # ot = (r * scaler) * num in one DVE op
                nc.vector.scalar_tensor_tensor(
                    out=ot[:, h * KH : (h + 1) * KH],
                    in0=d01[:],
                    scalar=scaler[:],
                    in1=e0[:],
                    op0=mybir.AluOpType.mult,
                    op1=mybir.AluOpType.mult,
                )
            # store on the ACT hwdge queue: keeps the SP queue (loads +
            # transposes) free and lets the next iteration's tile-0 load
            # start without queueing behind the last store
            nc.scalar.dma_start(out_d[row, :], ot[:])

        # prologue
        loaded = {0: emit_load(0)}
        conv = {0: emit_conv(0, loaded[0])}
        transposed = {0: emit_transp(0, conv[0][0])}
        if NPT > 1:
            loaded[1] = emit_load(1)

        prev = None
        for p in range(NPT):
            if p + 2 < NPT:
                loaded[p + 2] = emit_load(p + 2)
            if p + 1 < NPT:
                conv[p + 1] = emit_conv(p + 1, loaded[p + 1])
                transposed[p + 1] = emit_transp(p + 1, conv[p + 1][0])
            es = emit_pe(p, transposed.pop(p))
            muls_src = conv[p][0] if tail_bf16 else loaded[p]
            state = (p, muls_src, es, conv[p][1])
            if prev is not None and elem:
                emit_tail(prev)
            prev = state
        if elem:
            emit_tail(prev)

        if rep_cm is not None:
            rep_cm.__exit__(None, None, None)

    nc.compile()
    return nc


def _get_nc(repeat: int = 1, **flags):
    key = ("nc", repeat, tuple(sorted(flags.items())))
    if key not in _CACHE:
        _CACHE[key] = _build(repeat, **flags)
    return _CACHE[key]


def _run(inputs, trace=False):
    nc = _get_nc()
    w = np.ascontiguousarray(np.asarray(inputs["W"], dtype=np.float32))
    in_maps = []
    for c in range(N_CORES):
        sl = slice(c * B_SHARD, (c + 1) * B_SHARD)
        im = {"W": w}
        for m in range(NM):
            im[f"mod{m}"] = np.ascontiguousarray(
                np.asarray(inputs[f"mod{m}"], dtype=np.float32)[sl]
            )
        in_maps.append(im)
    return run_bass_kernel_spmd(
        nc, in_maps, core_ids=list(range(N_CORES)), trace=trace
    )


def kernel(**inputs) -> np.ndarray:
    res = _run(inputs, trace=False)
    return np.concatenate(
        [res.results[c]["out"] for c in range(N_CORES)], axis=0
    ).astype(np.float32)


# revision 27
# speedup vs baseline: 1.5458x; 1.5458x over previous
"""Trainium2 Bass kernel for the 4-modality attention-fusion module.

Computes, for full inputs mod0..mod3 [16384, 1024] f32 and W [1024, 1024] f32:
    scores_m = mod_m @ W.T                      (per modality)
    attn     = softmax over m of scores         (elementwise over [B, L])
    fused    = sum_m mod_m * attn_m
    scaler_b = 1 + #{m : sum_l mod_m[b, l] == 0}
    out      = fused * scaler[:, None]

Sharded data-parallel over 8 NeuronCores along the batch dim (2048 rows each),
W replicated. Design (PE runs ONLY the score matmuls):
  - W.T is built resident in SBUF in bf16 once, via XBAR DMA transposes
    (wt[p, j, k] = W[k, j*128 + p]);
  - per 128-patient tile, the f32 mod tiles are cast to bf16 on ACT; that
    same ACT op carries accum_out, producing the per-row sums (zero-modality
    detection) for free;
  - the bf16 mod tiles are transposed by the DMA XBAR engine (SBUF->SBUF,
    ~0.9us per [128,1024]) into the matmul stationary layout — the PE does
    no transposes and the ACT no PSUM evictions;
  - bf16 matmuls accumulate scores in PSUM (1 col/cycle like f32r, but with
    fast weight load and no PE transpose overhead);
  - softmax over the 4 modalities entirely on DVE: exp on ACT straight out
    of PSUM to bf16 tiles, numerator products/sums in bf16 (2x DVE mode),
    denominator accumulated to f32 for the fast-reciprocal op, zero-modality
    rescale folded into the final scalar_tensor_tensor.  Pool (gpsimd) is
    deliberately idle: its Add/Multiply run far below roofline on HW and
    became the critical path when given the denominator chain;
  - per-segment emission order keeps next-tile casts ahead of this tile's
    exps in the ACT queue and next-tile loads ahead of the XBAR transposes
    in the SP queue; modt/psum rings are deep enough (3 and 8 banks) that
    the PE never waits on stationaries or exp drains.

Measured on 8 axon trn2 cores via the R-repeat slope method: ~350 us per
iteration (baseline f32r/PE-transpose kernel: ~437 us by the harness's
measurement, ~358 us by this harness's method), max rel err ~7.5e-3.
"""

import sys

sys.path.insert(0, "/opt/trn_rl_repo")

from contextlib import ExitStack

import numpy as np

import concourse.bass as bass
import concourse.bacc as bacc
import concourse.mybir as mybir
import concourse.tile as tile
from concourse.bass_utils import run_bass_kernel_spmd

F32 = mybir.dt.float32
BF16 = mybir.dt.bfloat16
AF = mybir.ActivationFunctionType

N_CORES = 8
B_FULL = 16384
L = 1024
P = 128
B_SHARD = B_FULL // N_CORES          # 2048
NPT = B_SHARD // P                   # 16 patient tiles per core
NM = 4                               # modalities
NLC = L // P                         # 8 l-chunks (contraction)
NH = 2                               # k halves
KH = L // NH                         # 512

_CACHE: dict = {}


def _build(
    repeat: int = 1,
    *,
    elem: bool = True,
    transp: bool = True,
    tail_bf16: bool = True,
):
    E_DT = BF16 if tail_bf16 else F32
    nc = bacc.Bacc("TRN2", target_bir_lowering=False, debug=False)
    mods_d = [
        nc.dram_tensor(f"mod{m}", [B_SHARD, L], F32, kind="ExternalInput").ap()
        for m in range(NM)
    ]
    w_d = nc.dram_tensor("W", [L, L], F32, kind="ExternalInput").ap()
    out_d = nc.dram_tensor("out", [B_SHARD, L], F32, kind="ExternalOutput").ap()

    with tile.TileContext(nc) as tc, ExitStack() as ctx:
        wt_p = ctx.enter_context(tc.tile_pool(name="wt", bufs=1))
        wload_p = ctx.enter_context(tc.tile_pool(name="wload", bufs=2))
        mod_p = ctx.enter_context(
            tc.tile_pool(name="mod", bufs=3 if tail_bf16 else 4)
        )
        modb_p = ctx.enter_context(
            tc.tile_pool(name="modb", bufs=4 if tail_bf16 else 2)
        )
        modt_p = ctx.enter_context(tc.tile_pool(name="modt", bufs=3))
        e_p = ctx.enter_context(tc.tile_pool(name="e", bufs=4))
        rs_p = ctx.enter_context(tc.tile_pool(name="rs", bufs=4))
        tmp_p = ctx.enter_context(tc.tile_pool(name="tmp", bufs=2))
        out_p = ctx.enter_context(tc.tile_pool(name="outp", bufs=2))
        ps_q = ctx.enter_context(
            tc.tile_pool(name="ps_q", bufs=8, space=bass.MemorySpace.PSUM)
        )

        # ---- WT resident in SBUF (bf16): wt[p, j, k] = W[k, j*128 + p] ----
        wt = wt_p.tile([P, NLC, L], BF16, tag="wt")
        for kc in range(NLC):
            wk = wload_p.tile([P, L], F32, tag="wk")
            nc.sync.dma_start(wk[:], w_d[kc * P : (kc + 1) * P, :])
            wkb = wload_p.tile([P, L], BF16, tag="wkb")
            nc.scalar.copy(wkb[:], wk[:])
            nc.sync.dma_start_transpose(wt[:, :, kc * P : (kc + 1) * P], wkb[:])

        # ---------------- main loop, software-pipelined ----------------
        rep_cm = (
            tc.For_i(
                0,
                repeat,
                1,
                hint_engines=(
                    mybir.EngineType.PE,
                    mybir.EngineType.DVE,
                    mybir.EngineType.Activation,
                    mybir.EngineType.Pool,
                    mybir.EngineType.SP,
                ),
            )
            if repeat > 1
            else None
        )
        if rep_cm is not None:
            rep_cm.__enter__()

        def emit_load(p):
            row = slice(p * P, (p + 1) * P)
            mods = []
            for m in range(NM):
                mt = mod_p.tile([P, L], F32, tag=f"mod{m}")
                nc.sync.dma_start(mt[:], mods_d[m][row, :])
                mods.append(mt)
            return mods

        def emit_conv(p, mods):
            """f32 -> bf16 casts on ACT into ONE fused [P, 4*L] tile;
            accum_out rides along to produce the per-modality row sums
            (zero-modality detection) for free."""
            rsum = rs_p.tile([P, NM], F32, tag="rsum")
            modb = modb_p.tile([P, NM * L], BF16, tag="modb")
            for m in range(NM):
                nc.scalar.activation(
                    modb[:, m * L : (m + 1) * L],
                    mods[m][:],
                    AF.Copy,
                    accum_out=rsum[:, m : m + 1],
                )
            return modb, rsum

        def emit_transp(p, modb):
            """ONE XBAR transpose per patient tile (not 4): the fused
            [P, 4096] -> [P, 32, 128] transfer amortizes the per-instruction
            DMA-drain overhead the transpose deadlock-guard imposes."""
            if not transp:
                return None
            mT = modt_p.tile([P, NM * NLC, P], BF16, tag="modt")
            nc.sync.dma_start_transpose(mT[:], modb[:])
            return mT

        def emit_pe(p, modts):
            """Score matmuls + trailing exps for tile p.

            Modalities run in interleaved PAIRS: within each j, consecutive
            matmuls alternate between the two modalities' stationaries, so
            each LDWEIGHTS can target the PE's background weight buffer and
            load during the previous matmul's stream (the reorder window
            only pulls LDWEIGHTS ahead for a non-conflicting buffer).  The
            four open accumulation chains per pair plus the previous pair's
            draining banks exactly fill the 8 PSUM banks."""
            es = {}
            for pair in ((0, 1), (2, 3)):
                sqs = {}
                for m in pair:
                    for h in range(NH):
                        sq = ps_q.tile([P, KH], F32, tag="sq")
                        sqs[(m, h)] = sq
                for j in range(NLC):
                    for h in range(NH):
                        for m in pair:
                            lhsT = (
                                modts[:, m * NLC + j, :]
                                if modts is not None
                                else wt[:, j, 0:P]
                            )
                            nc.tensor.matmul(
                                sqs[(m, h)][:],
                                lhsT,
                                wt[:, j, h * KH : (h + 1) * KH],
                                start=(j == 0),
                                stop=(j == NLC - 1),
                            )
                if elem:
                    for m in pair:
                        for h in range(NH):
                            e = e_p.tile([P, KH], E_DT, tag=f"e{m}")
                            nc.scalar.activation(e[:], sqs[(m, h)][:], AF.Exp)
                            es[(m, h)] = e
            return es

        def emit_tail(state):
            """Softmax combine + output for tile p (lags one segment)."""
            p, mods, es, rsum = state
            row = slice(p * P, (p + 1) * P)
            zt = tmp_p.tile([P, NM], F32, tag="zt")
            zs = tmp_p.tile([P, 1], F32, tag="zs")
            nc.vector.tensor_scalar(
                out=zt[:],
                in0=rsum[:],
                scalar1=0.0,
                scalar2=None,
                op0=mybir.AluOpType.is_equal,
                op1=mybir.AluOpType.add,
                accum_out=zs[:],
            )
            scaler = tmp_p.tile([P, 1], F32, tag="scaler")
            nc.vector.tensor_scalar_add(scaler[:], zs[:], 1.0)
            ot = out_p.tile([P, L], F32, tag="ot")
            for h in range(NH):
                e0, e1, e2, e3 = (es[(m, h)] for m in range(NM))
                # all tail tensor ops on DVE: gpsimd (Pool) Add/Multiply run
                # far below roofline on HW (software Q7 implementation) and
                # became the critical path when loaded with the denominator
                d01 = tmp_p.tile([P, KH], F32, tag="d01")
                d23 = tmp_p.tile([P, KH], F32, tag="d23")
                nc.vector.tensor_add(d01[:], e0[:], e1[:])
                nc.vector.tensor_add(d23[:], e2[:], e3[:])
                nc.vector.tensor_add(d01[:], d01[:], d23[:])
                # numerator in place (same-engine queue, no WAR stall)
                for m in range(NM):
                    if tail_bf16:
                        src = mods[:, m * L + h * KH : m * L + (h + 1) * KH]
                    else:
                        src = mods[m][:, h * KH : (h + 1) * KH]
                    nc.vector.tensor_mul(
                        es[(m, h)][:], es[(m, h)][:], src
                    )
                nc.vector.tensor_add(e0[:], e0[:], e1[:])
                nc.vector.tensor_add(e2[:], e2[:], e3[:])
                nc.vector.reciprocal_approx_fast(out=d01[:], in_=d01[:])
                nc.vector.tensor_add(e0[:], e0[:], e2[:])
                # ot = (r * scaler) * num in one DVE op
                nc.vector.scalar_tensor_tensor(
                    out=ot[:, h * KH : (h + 1) * KH],
                    in0=d01[:],
                    scalar=scaler[:],
                    in1=e0[:],
                    op0=mybir.AluOpType.mult,
                    op1=mybir.AluOpType.mult,
                )
            nc.sync.dma_start(out_d[row, :], ot[:])

        # prologue
        loaded = {0: emit_load(0)}
        conv = {0: emit_conv(0, loaded[0])}
        transposed = {0: emit_transp(0, conv[0][0])}
        if NPT > 1:
            loaded[1] = emit_load(1)

        prev = None
        for p in range(NPT):
            if p + 2 < NPT:
                loaded[p + 2] = emit_load(p + 2)
            if p + 1 < NPT:
                conv[p + 1] = emit_conv(p + 1, loaded[p + 1])
                transposed[p + 1] = emit_transp(p + 1, conv[p + 1][0])
            es = emit_pe(p, transposed.pop(p))
            muls_src = conv[p][0] if tail_bf16 else loaded[p]
            state = (p, muls_src, es, conv[p][1])
            if prev is not None and elem:
                emit_tail(prev)
            prev = state
        if elem:
            emit_tail(prev)

        if rep_cm is not None:
            rep_cm.__exit__(None, None, None)

    nc.compile()
    return nc


def _get_nc(repeat: int = 1, **flags):
    key = ("nc", repeat, tuple(sorted(flags.items())))
    if key not in _CACHE:
        _CACHE[key] = _build(repeat, **flags)
    return _CACHE[key]


def _run(inputs, trace=False):
    nc = _get_nc()
    w = np.ascontiguousarray(np.asarray(inputs["W"], dtype=np.float32))
    in_maps = []
    for c in range(N_CORES):
        sl = slice(c * B_SHARD, (c + 1) * B_SHARD)
        im = {"W": w}
        for m in range(NM):
            im[f"mod{m}"] = np.ascontiguousarray(
                np.asarray(inputs[f"mod{m}"], dtype=np.float32)[sl]
            )
        in_maps.append(im)
    return run_bass_kernel_spmd(
        nc, in_maps, core_ids=list(range(N_CORES)), trace=trace
    )


def kernel(**inputs) -> np.ndarray:
    res = _run(inputs, trace=False)
    return np.concatenate(
        [res.results[c]["out"] for c in range(N_CORES)], axis=0
    ).astype(np.float32)


# revision 28
# speedup vs baseline: 1.5602x; 1.0093x over previous
"""Trainium2 Bass kernel for the 4-modality attention-fusion module.

Computes, for full inputs mod0..mod3 [16384, 1024] f32 and W [1024, 1024] f32:
    scores_m = mod_m @ W.T                      (per modality)
    attn     = softmax over m of scores         (elementwise over [B, L])
    fused    = sum_m mod_m * attn_m
    scaler_b = 1 + #{m : sum_l mod_m[b, l] == 0}
    out      = fused * scaler[:, None]

Sharded data-parallel over 8 NeuronCores along the batch dim (2048 rows each),
W replicated. Design (PE runs ONLY the score matmuls):
  - W.T is built resident in SBUF in bf16 once, via XBAR DMA transposes
    (wt[p, j, k] = W[k, j*128 + p]);
  - per 128-patient tile, the f32 mod tiles are cast to bf16 on ACT; that
    same ACT op carries accum_out, producing the per-row sums (zero-modality
    detection) for free;
  - the bf16 mod tiles are transposed by the DMA XBAR engine (SBUF->SBUF,
    ~0.9us per [128,1024]) into the matmul stationary layout — the PE does
    no transposes and the ACT no PSUM evictions;
  - bf16 matmuls accumulate scores in PSUM (1 col/cycle like f32r, but with
    fast weight load and no PE transpose overhead);
  - softmax over the 4 modalities entirely on DVE: exp on ACT straight out
    of PSUM to bf16 tiles, numerator products/sums in bf16 (2x DVE mode),
    denominator accumulated to f32 for the fast-reciprocal op, zero-modality
    rescale folded into the final scalar_tensor_tensor.  Pool (gpsimd) is
    deliberately idle: its Add/Multiply run far below roofline on HW and
    became the critical path when given the denominator chain;
  - per-segment emission order keeps next-tile casts ahead of this tile's
    exps in the ACT queue and next-tile loads ahead of the XBAR transposes
    in the SP queue; modt/psum rings are deep enough (3 and 8 banks) that
    the PE never waits on stationaries or exp drains.

Measured on 8 axon trn2 cores via the R-repeat slope method: ~350 us per
iteration (baseline f32r/PE-transpose kernel: ~437 us by the harness's
measurement, ~358 us by this harness's method), max rel err ~7.5e-3.
"""

import sys

sys.path.insert(0, "/opt/trn_rl_repo")

from contextlib import ExitStack

import numpy as np

import concourse.bass as bass
import concourse.bacc as bacc
import concourse.mybir as mybir
import concourse.tile as tile
from concourse.bass_utils import run_bass_kernel_spmd

F32 = mybir.dt.float32
BF16 = mybir.dt.bfloat16
AF = mybir.ActivationFunctionType

N_CORES = 8
B_FULL = 16384
L = 1024
P = 128
B_SHARD = B_FULL // N_CORES          # 2048
NPT = B_SHARD // P                   # 16 patient tiles per core
NM = 4                               # modalities
NLC = L // P                         # 8 l-chunks (contraction)
NH = 2                               # k halves
KH = L // NH                         # 512

_CACHE: dict = {}


def _build(
    repeat: int = 1,
    *,
    elem: bool = True,
    transp: bool = True,
    tail_bf16: bool = True,
):
    E_DT = BF16 if tail_bf16 else F32
    nc = bacc.Bacc("TRN2", target_bir_lowering=False, debug=False)
    mods_d = [
        nc.dram_tensor(f"mod{m}", [B_SHARD, L], F32, kind="ExternalInput").ap()
        for m in range(NM)
    ]
    w_d = nc.dram_tensor("W", [L, L], F32, kind="ExternalInput").ap()
    out_d = nc.dram_tensor("out", [B_SHARD, L], F32, kind="ExternalOutput").ap()

    with tile.TileContext(nc) as tc, ExitStack() as ctx:
        wt_p = ctx.enter_context(tc.tile_pool(name="wt", bufs=1))
        wload_p = ctx.enter_context(tc.tile_pool(name="wload", bufs=2))
        mod_p = ctx.enter_context(
            tc.tile_pool(name="mod", bufs=3 if tail_bf16 else 4)
        )
        modb_p = ctx.enter_context(
            tc.tile_pool(name="modb", bufs=4 if tail_bf16 else 2)
        )
        modt_p = ctx.enter_context(tc.tile_pool(name="modt", bufs=3))
        e_p = ctx.enter_context(tc.tile_pool(name="e", bufs=4))
        rs_p = ctx.enter_context(tc.tile_pool(name="rs", bufs=4))
        tmp_p = ctx.enter_context(tc.tile_pool(name="tmp", bufs=2))
        out_p = ctx.enter_context(tc.tile_pool(name="outp", bufs=2))
        ps_q = ctx.enter_context(
            tc.tile_pool(name="ps_q", bufs=8, space=bass.MemorySpace.PSUM)
        )

        # ---- WT resident in SBUF (bf16): wt[p, j, k] = W[k, j*128 + p] ----
        wt = wt_p.tile([P, NLC, L], BF16, tag="wt")
        for kc in range(NLC):
            wk = wload_p.tile([P, L], F32, tag="wk")
            nc.sync.dma_start(wk[:], w_d[kc * P : (kc + 1) * P, :])
            wkb = wload_p.tile([P, L], BF16, tag="wkb")
            nc.scalar.copy(wkb[:], wk[:])
            nc.sync.dma_start_transpose(wt[:, :, kc * P : (kc + 1) * P], wkb[:])

        # ---------------- main loop, software-pipelined ----------------
        rep_cm = (
            tc.For_i(
                0,
                repeat,
                1,
                hint_engines=(
                    mybir.EngineType.PE,
                    mybir.EngineType.DVE,
                    mybir.EngineType.Activation,
                    mybir.EngineType.Pool,
                    mybir.EngineType.SP,
                ),
            )
            if repeat > 1
            else None
        )
        if rep_cm is not None:
            rep_cm.__enter__()

        def emit_load(p):
            row = slice(p * P, (p + 1) * P)
            mods = []
            for m in range(NM):
                mt = mod_p.tile([P, L], F32, tag=f"mod{m}")
                nc.sync.dma_start(mt[:], mods_d[m][row, :])
                mods.append(mt)
            return mods

        def emit_conv(p, mods):
            """f32 -> bf16 casts on ACT into ONE fused [P, 4*L] tile;
            accum_out rides along to produce the per-modality row sums
            (zero-modality detection) for free."""
            rsum = rs_p.tile([P, NM], F32, tag="rsum")
            modb = modb_p.tile([P, NM * L], BF16, tag="modb")
            for m in range(NM):
                nc.scalar.activation(
                    modb[:, m * L : (m + 1) * L],
                    mods[m][:],
                    AF.Copy,
                    accum_out=rsum[:, m : m + 1],
                )
            return modb, rsum

        def emit_transp(p, modb):
            """ONE XBAR transpose per patient tile (not 4): the fused
            [P, 4096] -> [P, 32, 128] transfer amortizes the per-instruction
            DMA-drain overhead the transpose deadlock-guard imposes."""
            if not transp:
                return None
            mT = modt_p.tile([P, NM * NLC, P], BF16, tag="modt")
            nc.sync.dma_start_transpose(mT[:], modb[:])
            return mT

        def emit_pe(p, modts):
            """Score matmuls + trailing exps for tile p.

            Modalities run in interleaved PAIRS: within each j, consecutive
            matmuls alternate between the two modalities' stationaries, so
            each LDWEIGHTS can target the PE's background weight buffer and
            load during the previous matmul's stream (the reorder window
            only pulls LDWEIGHTS ahead for a non-conflicting buffer).  The
            four open accumulation chains per pair plus the previous pair's
            draining banks exactly fill the 8 PSUM banks."""
            es = {}
            for pair in ((0, 1), (2, 3)):
                sqs = {}
                for m in pair:
                    for h in range(NH):
                        sq = ps_q.tile([P, KH], F32, tag="sq")
                        sqs[(m, h)] = sq
                for j in range(NLC):
                    for h in range(NH):
                        for m in pair:
                            lhsT = (
                                modts[:, m * NLC + j, :]
                                if modts is not None
                                else wt[:, j, 0:P]
                            )
                            nc.tensor.matmul(
                                sqs[(m, h)][:],
                                lhsT,
                                wt[:, j, h * KH : (h + 1) * KH],
                                start=(j == 0),
                                stop=(j == NLC - 1),
                            )
                if elem:
                    for m in pair:
                        for h in range(NH):
                            e = e_p.tile([P, KH], E_DT, tag=f"e{m}")
                            nc.scalar.activation(e[:], sqs[(m, h)][:], AF.Exp)
                            es[(m, h)] = e
            return es

        def emit_tail(state):
            """Softmax combine + output for tile p (lags one segment)."""
            p, mods, es, rsum = state
            row = slice(p * P, (p + 1) * P)
            zt = tmp_p.tile([P, NM], F32, tag="zt")
            zs = tmp_p.tile([P, 1], F32, tag="zs")
            nc.vector.tensor_scalar(
                out=zt[:],
                in0=rsum[:],
                scalar1=0.0,
                scalar2=None,
                op0=mybir.AluOpType.is_equal,
                op1=mybir.AluOpType.add,
                accum_out=zs[:],
            )
            scaler = tmp_p.tile([P, 1], F32, tag="scaler")
            nc.vector.tensor_scalar_add(scaler[:], zs[:], 1.0)
            ot = out_p.tile([P, L], F32, tag="ot")
            for h in range(NH):
                e0, e1, e2, e3 = (es[(m, h)] for m in range(NM))
                # all tail tensor ops on DVE: gpsimd (Pool) Add/Multiply run
                # far below roofline on HW (software Q7 implementation) and
                # became the critical path when loaded with the denominator
                d01 = tmp_p.tile([P, KH], F32, tag="d01")
                d23 = tmp_p.tile([P, KH], F32, tag="d23")
                nc.vector.tensor_add(d01[:], e0[:], e1[:])
                nc.vector.tensor_add(d23[:], e2[:], e3[:])
                nc.vector.tensor_add(d01[:], d01[:], d23[:])
                # numerator in place (same-engine queue, no WAR stall)
                for m in range(NM):
                    if tail_bf16:
                        src = mods[:, m * L + h * KH : m * L + (h + 1) * KH]
                    else:
                        src = mods[m][:, h * KH : (h + 1) * KH]
                    nc.vector.tensor_mul(
                        es[(m, h)][:], es[(m, h)][:], src
                    )
                nc.vector.tensor_add(e0[:], e0[:], e1[:])
                nc.vector.tensor_add(e2[:], e2[:], e3[:])
                nc.vector.reciprocal_approx_fast(out=d01[:], in_=d01[:])
                nc.vector.tensor_add(e0[:], e0[:], e2[:])
                # ot = (r * scaler) * num in one DVE op
                nc.vector.scalar_tensor_tensor(
                    out=ot[:, h * KH : (h + 1) * KH],
                    in0=d01[:],
                    scalar=scaler[:],
                    in1=e0[:],
                    op0=mybir.AluOpType.mult,
                    op1=mybir.AluOpType.mult,
                )
            # store via gpsimd SWDGE: keeps the SP queue pure loads+transposes,
            # so the next iteration's tile-0 load chain isn't queued behind
            # the last tile's softmax tail (kills the per-iteration epilogue
            # bubble).  Pool only generates descriptors here — its slow
            # compute ALUs stay out of the critical path.
            nc.gpsimd.dma_start(out_d[row, :], ot[:])

        # prologue
        loaded = {0: emit_load(0)}
        conv = {0: emit_conv(0, loaded[0])}
        transposed = {0: emit_transp(0, conv[0][0])}
        if NPT > 1:
            loaded[1] = emit_load(1)

        prev = None
        for p in range(NPT):
            if p + 2 < NPT:
                loaded[p + 2] = emit_load(p + 2)
            if p + 1 < NPT:
                conv[p + 1] = emit_conv(p + 1, loaded[p + 1])
                transposed[p + 1] = emit_transp(p + 1, conv[p + 1][0])
            es = emit_pe(p, transposed.pop(p))
            muls_src = conv[p][0] if tail_bf16 else loaded[p]
            state = (p, muls_src, es, conv[p][1])
            if prev is not None and elem:
                emit_tail(prev)
            prev = state
        if elem:
            emit_tail(prev)

        if rep_cm is not None:
            rep_cm.__exit__(None, None, None)

    nc.compile()
    return nc


def _get_nc(repeat: int = 1, **flags):
    key = ("nc", repeat, tuple(sorted(flags.items())))
    if key not in _CACHE:
        _CACHE[key] = _build(repeat, **flags)
    return _CACHE[key]


def _run(inputs, trace=False):
    nc = _get_nc()
    w = np.ascontiguousarray(np.asarray(inputs["W"], dtype=np.float32))
    in_maps = []
    for c in range(N_CORES):
        sl = slice(c * B_SHARD, (c + 1) * B_SHARD)
        im = {"W": w}
        for m in range(NM):
            im[f"mod{m}"] = np.ascontiguousarray(
                np.asarray(inputs[f"mod{m}"], dtype=np.float32)[sl]
            )
        in_maps.append(im)
    return run_bass_kernel_spmd(
        nc, in_maps, core_ids=list(range(N_CORES)), trace=trace
    )


def kernel(**inputs) -> np.ndarray:
    res = _run(inputs, trace=False)
    return np.concatenate(
        [res.results[c]["out"] for c in range(N_CORES)], axis=0
    ).astype(np.float32)


# revision 31
# speedup vs baseline: 1.6589x; 1.0633x over previous
"""Trainium2 Bass kernel for the 4-modality attention-fusion module.

Computes, for full inputs mod0..mod3 [16384, 1024] f32 and W [1024, 1024] f32:
    scores_m = mod_m @ W.T                      (per modality)
    attn     = softmax over m of scores         (elementwise over [B, L])
    fused    = sum_m mod_m * attn_m
    scaler_b = 1 + #{m : sum_l mod_m[b, l] == 0}
    out      = fused * scaler[:, None]

Sharded data-parallel over 8 NeuronCores along the batch dim (2048 rows each),
W replicated. Design (PE runs ONLY the score matmuls):
  - W.T is built resident in SBUF in bf16 once, via XBAR DMA transposes
    (wt[p, j, k] = W[k, j*128 + p]);
  - per 128-patient tile, the f32 mod tiles are cast to bf16 on ACT; that
    same ACT op carries accum_out, producing the per-row sums (zero-modality
    detection) for free;
  - the bf16 mod tiles are transposed by the DMA XBAR engine (SBUF->SBUF,
    ~0.9us per [128,1024]) into the matmul stationary layout — the PE does
    no transposes and the ACT no PSUM evictions;
  - bf16 matmuls accumulate scores in PSUM (1 col/cycle like f32r, but with
    fast weight load and no PE transpose overhead);
  - softmax over the 4 modalities entirely on DVE: exp on ACT straight out
    of PSUM to bf16 tiles, numerator products/sums in bf16 (2x DVE mode),
    denominator accumulated to f32 for the fast-reciprocal op, zero-modality
    rescale folded into the final scalar_tensor_tensor.  Pool (gpsimd) is
    deliberately idle: its Add/Multiply run far below roofline on HW and
    became the critical path when given the denominator chain;
  - per-segment emission order keeps next-tile casts ahead of this tile's
    exps in the ACT queue and next-tile loads ahead of the XBAR transposes
    in the SP queue; modt/psum rings are deep enough (3 and 8 banks) that
    the PE never waits on stationaries or exp drains.

Measured on 8 axon trn2 cores via the R-repeat slope method: ~350 us per
iteration (baseline f32r/PE-transpose kernel: ~437 us by the harness's
measurement, ~358 us by this harness's method), max rel err ~7.5e-3.
"""

import sys

sys.path.insert(0, "/opt/trn_rl_repo")

from contextlib import ExitStack

import numpy as np

import concourse.bass as bass
import concourse.bacc as bacc
import concourse.mybir as mybir
import concourse.tile as tile
from concourse.bass_utils import run_bass_kernel_spmd

F32 = mybir.dt.float32
BF16 = mybir.dt.bfloat16
AF = mybir.ActivationFunctionType

N_CORES = 8
B_FULL = 16384
L = 1024
P = 128
B_SHARD = B_FULL // N_CORES          # 2048
NPT = B_SHARD // P                   # 16 patient tiles per core
NM = 4                               # modalities
NLC = L // P                         # 8 l-chunks (contraction)
NH = 2                               # k halves
KH = L // NH                         # 512

_CACHE: dict = {}


def _build(
    repeat: int = 1,
    *,
    elem: bool = True,
    transp: bool = True,
    tail_bf16: bool = True,
):
    E_DT = BF16 if tail_bf16 else F32
    nc = bacc.Bacc("TRN2", target_bir_lowering=False, debug=False)
    mods_d = [
        nc.dram_tensor(f"mod{m}", [B_SHARD, L], F32, kind="ExternalInput").ap()
        for m in range(NM)
    ]
    w_d = nc.dram_tensor("W", [L, L], F32, kind="ExternalInput").ap()
    out_d = nc.dram_tensor("out", [B_SHARD, L], F32, kind="ExternalOutput").ap()

    with tile.TileContext(nc) as tc, ExitStack() as ctx:
        wt_p = ctx.enter_context(tc.tile_pool(name="wt", bufs=1))
        wload_p = ctx.enter_context(tc.tile_pool(name="wload", bufs=2))
        mod_p = ctx.enter_context(
            tc.tile_pool(name="mod", bufs=3 if tail_bf16 else 4)
        )
        modb_p = ctx.enter_context(
            tc.tile_pool(name="modb", bufs=4 if tail_bf16 else 2)
        )
        modt_p = ctx.enter_context(tc.tile_pool(name="modt", bufs=3))
        e_p = ctx.enter_context(tc.tile_pool(name="e", bufs=4))
        rs_p = ctx.enter_context(tc.tile_pool(name="rs", bufs=4))
        tmp_p = ctx.enter_context(tc.tile_pool(name="tmp", bufs=2))
        out_p = ctx.enter_context(tc.tile_pool(name="outp", bufs=2))
        ps_q = ctx.enter_context(
            tc.tile_pool(name="ps_q", bufs=8, space=bass.MemorySpace.PSUM)
        )

        # ---- WT resident in SBUF (bf16): wt[p, j, k] = W[k, j*128 + p] ----
        wt = wt_p.tile([P, NLC, L], BF16, tag="wt")
        for kc in range(NLC):
            wk = wload_p.tile([P, L], F32, tag="wk")
            nc.sync.dma_start(wk[:], w_d[kc * P : (kc + 1) * P, :])
            wkb = wload_p.tile([P, L], BF16, tag="wkb")
            nc.scalar.copy(wkb[:], wk[:])
            nc.sync.dma_start_transpose(wt[:, :, kc * P : (kc + 1) * P], wkb[:])

        # ---------------- main loop, software-pipelined ----------------
        rep_cm = (
            tc.For_i(
                0,
                repeat,
                1,
                hint_engines=(
                    mybir.EngineType.PE,
                    mybir.EngineType.DVE,
                    mybir.EngineType.Activation,
                    mybir.EngineType.Pool,
                    mybir.EngineType.SP,
                ),
            )
            if repeat > 1
            else None
        )
        # NOTE: entered AFTER the prologue emission below, so the prologue
        # runs once and each loop pass gets its tile-0/1 prep from the
        # previous pass's rotated-prologue emission.

        def emit_load(p):
            row = slice(p * P, (p + 1) * P)
            mods = []
            for m in range(NM):
                mt = mod_p.tile([P, L], F32, tag=f"mod{m}")
                nc.sync.dma_start(mt[:], mods_d[m][row, :])
                mods.append(mt)
            return mods

        def emit_conv(p, mods):
            """f32 -> bf16 casts on ACT into ONE fused [P, 4*L] tile;
            accum_out rides along to produce the per-modality row sums
            (zero-modality detection) for free."""
            rsum = rs_p.tile([P, NM], F32, tag="rsum")
            modb = modb_p.tile([P, NM * L], BF16, tag="modb")
            for m in range(NM):
                nc.scalar.activation(
                    modb[:, m * L : (m + 1) * L],
                    mods[m][:],
                    AF.Copy,
                    accum_out=rsum[:, m : m + 1],
                )
            return modb, rsum

        def emit_transp(p, modb):
            """ONE XBAR transpose per patient tile (not 4): the fused
            [P, 4096] -> [P, 32, 128] transfer amortizes the per-instruction
            DMA-drain overhead the transpose deadlock-guard imposes."""
            if not transp:
                return None
            mT = modt_p.tile([P, NM * NLC, P], BF16, tag="modt")
            nc.sync.dma_start_transpose(mT[:], modb[:])
            return mT

        def emit_pe(p, modts):
            """Score matmuls + trailing exps for tile p.

            Modalities run in interleaved PAIRS: within each j, consecutive
            matmuls alternate between the two modalities' stationaries, so
            each LDWEIGHTS can target the PE's background weight buffer and
            load during the previous matmul's stream (the reorder window
            only pulls LDWEIGHTS ahead for a non-conflicting buffer).  The
            four open accumulation chains per pair plus the previous pair's
            draining banks exactly fill the 8 PSUM banks."""
            es = {}
            for pair in ((0, 1), (2, 3)):
                sqs = {}
                for m in pair:
                    for h in range(NH):
                        sq = ps_q.tile([P, KH], F32, tag="sq")
                        sqs[(m, h)] = sq
                for j in range(NLC):
                    for h in range(NH):
                        for m in pair:
                            lhsT = (
                                modts[:, m * NLC + j, :]
                                if modts is not None
                                else wt[:, j, 0:P]
                            )
                            nc.tensor.matmul(
                                sqs[(m, h)][:],
                                lhsT,
                                wt[:, j, h * KH : (h + 1) * KH],
                                start=(j == 0),
                                stop=(j == NLC - 1),
                            )
                if elem:
                    for m in pair:
                        for h in range(NH):
                            e = e_p.tile([P, KH], E_DT, tag=f"e{m}")
                            nc.scalar.activation(e[:], sqs[(m, h)][:], AF.Exp)
                            es[(m, h)] = e
            return es

        def emit_tail(state):
            """Softmax combine + output for tile p (lags one segment)."""
            p, mods, es, rsum = state
            row = slice(p * P, (p + 1) * P)
            zt = tmp_p.tile([P, NM], F32, tag="zt")
            zs = tmp_p.tile([P, 1], F32, tag="zs")
            nc.vector.tensor_scalar(
                out=zt[:],
                in0=rsum[:],
                scalar1=0.0,
                scalar2=None,
                op0=mybir.AluOpType.is_equal,
                op1=mybir.AluOpType.add,
                accum_out=zs[:],
            )
            scaler = tmp_p.tile([P, 1], F32, tag="scaler")
            nc.vector.tensor_scalar_add(scaler[:], zs[:], 1.0)
            ot = out_p.tile([P, L], F32, tag="ot")
            for h in range(NH):
                e0, e1, e2, e3 = (es[(m, h)] for m in range(NM))
                # all tail tensor ops on DVE: gpsimd (Pool) Add/Multiply run
                # far below roofline on HW (software Q7 implementation) and
                # became the critical path when loaded with the denominator
                d01 = tmp_p.tile([P, KH], F32, tag="d01")
                d23 = tmp_p.tile([P, KH], F32, tag="d23")
                nc.vector.tensor_add(d01[:], e0[:], e1[:])
                nc.vector.tensor_add(d23[:], e2[:], e3[:])
                nc.vector.tensor_add(d01[:], d01[:], d23[:])
                # numerator in place (same-engine queue, no WAR stall)
                for m in range(NM):
                    if tail_bf16:
                        src = mods[:, m * L + h * KH : m * L + (h + 1) * KH]
                    else:
                        src = mods[m][:, h * KH : (h + 1) * KH]
                    nc.vector.tensor_mul(
                        es[(m, h)][:], es[(m, h)][:], src
                    )
                nc.vector.tensor_add(e0[:], e0[:], e1[:])
                nc.vector.tensor_add(e2[:], e2[:], e3[:])
                nc.vector.reciprocal_approx_fast(out=d01[:], in_=d01[:])
                nc.vector.tensor_add(e0[:], e0[:], e2[:])
                # ot = (r * scaler) * num in one DVE op
                nc.vector.scalar_tensor_tensor(
                    out=ot[:, h * KH : (h + 1) * KH],
                    in0=d01[:],
                    scalar=scaler[:],
                    in1=e0[:],
                    op0=mybir.AluOpType.mult,
                    op1=mybir.AluOpType.mult,
                )
            # store via gpsimd SWDGE: keeps the SP queue pure loads+transposes,
            # so the next iteration's tile-0 load chain isn't queued behind
            # the last tile's softmax tail (kills the per-iteration epilogue
            # bubble).  Pool only generates descriptors here — its slow
            # compute ALUs stay out of the critical path.
            nc.gpsimd.dma_start(out_d[row, :], ot[:])

        # prologue
        loaded = {0: emit_load(0)}
        conv = {0: emit_conv(0, loaded[0])}
        transposed = {0: emit_transp(0, conv[0][0])}
        if NPT > 1:
            loaded[1] = emit_load(1)
        # kept so the rotated prologue can refresh the SAME tiles each pass
        pro_loaded0, pro_conv0 = loaded[0], conv[0]
        pro_transp0, pro_loaded1 = transposed[0], loaded[1]

        if rep_cm is not None:
            rep_cm.__enter__()

        def emit_rotated_prologue():
            """Re-emitted at the END of each repeat-loop pass, BEFORE the
            last tile's PE segment: refreshes tile 0/1 prep for the NEXT
            pass into the same tile objects the pass-start consumers read
            (loop-carried RAW through the For_i back-edge).  Without this,
            the next pass's tile-0 cast queues on ACT behind exps(15) and
            the PE idles ~10us per pass for the load->cast->transpose
            chain."""
            for m in range(NM):
                nc.sync.dma_start(pro_loaded0[m][:], mods_d[m][0:P, :])
            for m in range(NM):
                nc.scalar.activation(
                    pro_conv0[0][:, m * L : (m + 1) * L],
                    pro_loaded0[m][:],
                    AF.Copy,
                    accum_out=pro_conv0[1][:, m : m + 1],
                )
            if transp:
                nc.sync.dma_start_transpose(pro_transp0[:], pro_conv0[0][:])
            for m in range(NM):
                nc.sync.dma_start(pro_loaded1[m][:], mods_d[m][P : 2 * P, :])

        prev = None
        for p in range(NPT):
            if p + 2 < NPT:
                loaded[p + 2] = emit_load(p + 2)
            if p + 1 < NPT:
                conv[p + 1] = emit_conv(p + 1, loaded[p + 1])
                transposed[p + 1] = emit_transp(p + 1, conv[p + 1][0])
            if p == NPT - 1 and rep_cm is not None:
                emit_rotated_prologue()
            es = emit_pe(p, transposed.pop(p))
            muls_src = conv[p][0] if tail_bf16 else loaded[p]
            state = (p, muls_src, es, conv[p][1])
            if prev is not None and elem:
                emit_tail(prev)
            prev = state
        if elem:
            emit_tail(prev)

        if rep_cm is not None:
            rep_cm.__exit__(None, None, None)

    nc.compile()
    return nc


def _get_nc(repeat: int = 1, **flags):
    key = ("nc", repeat, tuple(sorted(flags.items())))
    if key not in _CACHE:
        _CACHE[key] = _build(repeat, **flags)
    return _CACHE[key]


def _run(inputs, trace=False):
    nc = _get_nc()
    w = np.ascontiguousarray(np.asarray(inputs["W"], dtype=np.float32))
    in_maps = []
    for c in range(N_CORES):
        sl = slice(c * B_SHARD, (c + 1) * B_SHARD)
        im = {"W": w}
        for m in range(NM):
            im[f"mod{m}"] = np.ascontiguousarray(
                np.asarray(inputs[f"mod{m}"], dtype=np.float32)[sl]
            )
        in_maps.append(im)
    return run_bass_kernel_spmd(
        nc, in_maps, core_ids=list(range(N_CORES)), trace=trace
    )


def kernel(**inputs) -> np.ndarray:
    res = _run(inputs, trace=False)
    return np.concatenate(
        [res.results[c]["out"] for c in range(N_CORES)], axis=0
    ).astype(np.float32)


# revision 32
# speedup vs baseline: 1.6804x; 1.0129x over previous
"""Trainium2 Bass kernel for the 4-modality attention-fusion module.

Computes, for full inputs mod0..mod3 [16384, 1024] f32 and W [1024, 1024] f32:
    scores_m = mod_m @ W.T                      (per modality)
    attn     = softmax over m of scores         (elementwise over [B, L])
    fused    = sum_m mod_m * attn_m
    scaler_b = 1 + #{m : sum_l mod_m[b, l] == 0}
    out      = fused * scaler[:, None]

Sharded data-parallel over 8 NeuronCores along the batch dim (2048 rows each),
W replicated. Design (PE runs ONLY the score matmuls):
  - W.T is built resident in SBUF in bf16 once, via XBAR DMA transposes
    (wt[p, j, k] = W[k, j*128 + p]);
  - per 128-patient tile, the f32 mod tiles are cast to bf16 on ACT; that
    same ACT op carries accum_out, producing the per-row sums (zero-modality
    detection) for free;
  - the bf16 mod tiles are transposed by the DMA XBAR engine (SBUF->SBUF,
    ~0.9us per [128,1024]) into the matmul stationary layout — the PE does
    no transposes and the ACT no PSUM evictions;
  - bf16 matmuls accumulate scores in PSUM (1 col/cycle like f32r, but with
    fast weight load and no PE transpose overhead);
  - softmax over the 4 modalities entirely on DVE: exp on ACT straight out
    of PSUM to bf16 tiles, numerator products/sums in bf16 (2x DVE mode),
    denominator accumulated to f32 for the fast-reciprocal op, zero-modality
    rescale folded into the final scalar_tensor_tensor.  Pool (gpsimd) is
    deliberately idle: its Add/Multiply run far below roofline on HW and
    became the critical path when given the denominator chain;
  - per-segment emission order keeps next-tile casts ahead of this tile's
    exps in the ACT queue and next-tile loads ahead of the XBAR transposes
    in the SP queue; modt/psum rings are deep enough (3 and 8 banks) that
    the PE never waits on stationaries or exp drains.

Measured on 8 axon trn2 cores via the R-repeat slope method: ~350 us per
iteration (baseline f32r/PE-transpose kernel: ~437 us by the harness's
measurement, ~358 us by this harness's method), max rel err ~7.5e-3.
"""

import sys

sys.path.insert(0, "/opt/trn_rl_repo")

from contextlib import ExitStack

import numpy as np

import concourse.bass as bass
import concourse.bacc as bacc
import concourse.mybir as mybir
import concourse.tile as tile
from concourse.bass_utils import run_bass_kernel_spmd

F32 = mybir.dt.float32
BF16 = mybir.dt.bfloat16
AF = mybir.ActivationFunctionType

N_CORES = 8
B_FULL = 16384
L = 1024
P = 128
B_SHARD = B_FULL // N_CORES          # 2048
NPT = B_SHARD // P                   # 16 patient tiles per core
NM = 4                               # modalities
NLC = L // P                         # 8 l-chunks (contraction)
NH = 2                               # k halves
KH = L // NH                         # 512

_CACHE: dict = {}


def _build(
    repeat: int = 1,
    *,
    elem: bool = True,
    transp: bool = True,
    tail_bf16: bool = True,
):
    E_DT = BF16 if tail_bf16 else F32
    nc = bacc.Bacc("TRN2", target_bir_lowering=False, debug=False)
    mods_d = [
        nc.dram_tensor(f"mod{m}", [B_SHARD, L], F32, kind="ExternalInput").ap()
        for m in range(NM)
    ]
    w_d = nc.dram_tensor("W", [L, L], F32, kind="ExternalInput").ap()
    out_d = nc.dram_tensor("out", [B_SHARD, L], F32, kind="ExternalOutput").ap()

    with tile.TileContext(nc) as tc, ExitStack() as ctx:
        wt_p = ctx.enter_context(tc.tile_pool(name="wt", bufs=1))
        wload_p = ctx.enter_context(tc.tile_pool(name="wload", bufs=2))
        mod_p = ctx.enter_context(
            tc.tile_pool(name="mod", bufs=3 if tail_bf16 else 4)
        )
        modb_p = ctx.enter_context(
            tc.tile_pool(name="modb", bufs=4 if tail_bf16 else 2)
        )
        modt_p = ctx.enter_context(tc.tile_pool(name="modt", bufs=3))
        e_p = ctx.enter_context(tc.tile_pool(name="e", bufs=6))
        rs_p = ctx.enter_context(tc.tile_pool(name="rs", bufs=4))
        tmp_p = ctx.enter_context(tc.tile_pool(name="tmp", bufs=3))
        out_p = ctx.enter_context(tc.tile_pool(name="outp", bufs=3))
        ps_q = ctx.enter_context(
            tc.tile_pool(name="ps_q", bufs=8, space=bass.MemorySpace.PSUM)
        )

        # ---- WT resident in SBUF (bf16): wt[p, j, k] = W[k, j*128 + p] ----
        wt = wt_p.tile([P, NLC, L], BF16, tag="wt")
        for kc in range(NLC):
            wk = wload_p.tile([P, L], F32, tag="wk")
            nc.sync.dma_start(wk[:], w_d[kc * P : (kc + 1) * P, :])
            wkb = wload_p.tile([P, L], BF16, tag="wkb")
            nc.scalar.copy(wkb[:], wk[:])
            nc.sync.dma_start_transpose(wt[:, :, kc * P : (kc + 1) * P], wkb[:])

        # ---------------- main loop, software-pipelined ----------------
        rep_cm = (
            tc.For_i(
                0,
                repeat,
                1,
                hint_engines=(
                    mybir.EngineType.PE,
                    mybir.EngineType.DVE,
                    mybir.EngineType.Activation,
                    mybir.EngineType.Pool,
                    mybir.EngineType.SP,
                ),
            )
            if repeat > 1
            else None
        )
        # NOTE: entered AFTER the prologue emission below, so the prologue
        # runs once and each loop pass gets its tile-0/1 prep from the
        # previous pass's rotated-prologue emission.

        def emit_load(p):
            row = slice(p * P, (p + 1) * P)
            mods = []
            for m in range(NM):
                mt = mod_p.tile([P, L], F32, tag=f"mod{m}")
                nc.sync.dma_start(mt[:], mods_d[m][row, :])
                mods.append(mt)
            return mods

        def emit_conv(p, mods):
            """f32 -> bf16 casts on ACT into ONE fused [P, 4*L] tile;
            accum_out rides along to produce the per-modality row sums
            (zero-modality detection) for free."""
            rsum = rs_p.tile([P, NM], F32, tag="rsum")
            modb = modb_p.tile([P, NM * L], BF16, tag="modb")
            for m in range(NM):
                nc.scalar.activation(
                    modb[:, m * L : (m + 1) * L],
                    mods[m][:],
                    AF.Copy,
                    accum_out=rsum[:, m : m + 1],
                )
            return modb, rsum

        def emit_transp(p, modb):
            """ONE XBAR transpose per patient tile (not 4): the fused
            [P, 4096] -> [P, 32, 128] transfer amortizes the per-instruction
            DMA-drain overhead the transpose deadlock-guard imposes."""
            if not transp:
                return None
            mT = modt_p.tile([P, NM * NLC, P], BF16, tag="modt")
            nc.sync.dma_start_transpose(mT[:], modb[:])
            return mT

        def emit_pe(p, modts):
            """Score matmuls + trailing exps for tile p.

            Modalities run in interleaved PAIRS: within each j, consecutive
            matmuls alternate between the two modalities' stationaries, so
            each LDWEIGHTS can target the PE's background weight buffer and
            load during the previous matmul's stream (the reorder window
            only pulls LDWEIGHTS ahead for a non-conflicting buffer).  The
            four open accumulation chains per pair plus the previous pair's
            draining banks exactly fill the 8 PSUM banks."""
            es = {}
            for pair in ((0, 1), (2, 3)):
                sqs = {}
                for m in pair:
                    for h in range(NH):
                        sq = ps_q.tile([P, KH], F32, tag="sq")
                        sqs[(m, h)] = sq
                for j in range(NLC):
                    for h in range(NH):
                        for m in pair:
                            lhsT = (
                                modts[:, m * NLC + j, :]
                                if modts is not None
                                else wt[:, j, 0:P]
                            )
                            nc.tensor.matmul(
                                sqs[(m, h)][:],
                                lhsT,
                                wt[:, j, h * KH : (h + 1) * KH],
                                start=(j == 0),
                                stop=(j == NLC - 1),
                            )
                if elem:
                    for m in pair:
                        for h in range(NH):
                            e = e_p.tile([P, KH], E_DT, tag=f"e{m}")
                            nc.scalar.activation(e[:], sqs[(m, h)][:], AF.Exp)
                            es[(m, h)] = e
            return es

        def emit_tail(state):
            """Softmax combine + output for tile p (lags one segment)."""
            p, mods, es, rsum = state
            row = slice(p * P, (p + 1) * P)
            zt = tmp_p.tile([P, NM], F32, tag="zt")
            zs = tmp_p.tile([P, 1], F32, tag="zs")
            nc.vector.tensor_scalar(
                out=zt[:],
                in0=rsum[:],
                scalar1=0.0,
                scalar2=None,
                op0=mybir.AluOpType.is_equal,
                op1=mybir.AluOpType.add,
                accum_out=zs[:],
            )
            scaler = tmp_p.tile([P, 1], F32, tag="scaler")
            nc.vector.tensor_scalar_add(scaler[:], zs[:], 1.0)
            ot = out_p.tile([P, L], F32, tag="ot")
            for h in range(NH):
                e0, e1, e2, e3 = (es[(m, h)] for m in range(NM))
                # all tail tensor ops on DVE: gpsimd (Pool) Add/Multiply run
                # far below roofline on HW (software Q7 implementation) and
                # became the critical path when loaded with the denominator
                d01 = tmp_p.tile([P, KH], F32, tag="d01")
                d23 = tmp_p.tile([P, KH], F32, tag="d23")
                nc.vector.tensor_add(d01[:], e0[:], e1[:])
                nc.vector.tensor_add(d23[:], e2[:], e3[:])
                nc.vector.tensor_add(d01[:], d01[:], d23[:])
                # numerator in place (same-engine queue, no WAR stall)
                for m in range(NM):
                    if tail_bf16:
                        src = mods[:, m * L + h * KH : m * L + (h + 1) * KH]
                    else:
                        src = mods[m][:, h * KH : (h + 1) * KH]
                    nc.vector.tensor_mul(
                        es[(m, h)][:], es[(m, h)][:], src
                    )
                nc.vector.tensor_add(e0[:], e0[:], e1[:])
                nc.vector.tensor_add(e2[:], e2[:], e3[:])
                nc.vector.reciprocal_approx_fast(out=d01[:], in_=d01[:])
                nc.vector.tensor_add(e0[:], e0[:], e2[:])
                # ot = (r * scaler) * num in one DVE op
                nc.vector.scalar_tensor_tensor(
                    out=ot[:, h * KH : (h + 1) * KH],
                    in0=d01[:],
                    scalar=scaler[:],
                    in1=e0[:],
                    op0=mybir.AluOpType.mult,
                    op1=mybir.AluOpType.mult,
                )
            # store via gpsimd SWDGE: keeps the SP queue pure loads+transposes,
            # so the next iteration's tile-0 load chain isn't queued behind
            # the last tile's softmax tail (kills the per-iteration epilogue
            # bubble).  Pool only generates descriptors here — its slow
            # compute ALUs stay out of the critical path.
            nc.gpsimd.dma_start(out_d[row, :], ot[:])

        # prologue
        loaded = {0: emit_load(0)}
        conv = {0: emit_conv(0, loaded[0])}
        transposed = {0: emit_transp(0, conv[0][0])}
        if NPT > 1:
            loaded[1] = emit_load(1)
        # kept so the rotated prologue can refresh the SAME tiles each pass
        pro_loaded0, pro_conv0 = loaded[0], conv[0]
        pro_transp0, pro_loaded1 = transposed[0], loaded[1]

        if rep_cm is not None:
            rep_cm.__enter__()

        def emit_rotated_prologue():
            """Re-emitted at the END of each repeat-loop pass, BEFORE the
            last tile's PE segment: refreshes tile 0/1 prep for the NEXT
            pass into the same tile objects the pass-start consumers read
            (loop-carried RAW through the For_i back-edge).  Without this,
            the next pass's tile-0 cast queues on ACT behind exps(15) and
            the PE idles ~10us per pass for the load->cast->transpose
            chain."""
            for m in range(NM):
                nc.sync.dma_start(pro_loaded0[m][:], mods_d[m][0:P, :])
            for m in range(NM):
                nc.scalar.activation(
                    pro_conv0[0][:, m * L : (m + 1) * L],
                    pro_loaded0[m][:],
                    AF.Copy,
                    accum_out=pro_conv0[1][:, m : m + 1],
                )
            if transp:
                nc.sync.dma_start_transpose(pro_transp0[:], pro_conv0[0][:])
            for m in range(NM):
                nc.sync.dma_start(pro_loaded1[m][:], mods_d[m][P : 2 * P, :])

        prev = None
        for p in range(NPT):
            if p + 2 < NPT:
                loaded[p + 2] = emit_load(p + 2)
            if p + 1 < NPT:
                conv[p + 1] = emit_conv(p + 1, loaded[p + 1])
                transposed[p + 1] = emit_transp(p + 1, conv[p + 1][0])
            if p == NPT - 1 and rep_cm is not None:
                emit_rotated_prologue()
            es = emit_pe(p, transposed.pop(p))
            muls_src = conv[p][0] if tail_bf16 else loaded[p]
            state = (p, muls_src, es, conv[p][1])
            if prev is not None and elem:
                emit_tail(prev)
            prev = state
        if elem:
            emit_tail(prev)

        if rep_cm is not None:
            rep_cm.__exit__(None, None, None)

    nc.compile()
    return nc


def _get_nc(repeat: int = 1, **flags):
    key = ("nc", repeat, tuple(sorted(flags.items())))
    if key not in _CACHE:
        _CACHE[key] = _build(repeat, **flags)
    return _CACHE[key]


def _run(inputs, trace=False):
    nc = _get_nc()
    w = np.ascontiguousarray(np.asarray(inputs["W"], dtype=np.float32))
    in_maps = []
    for c in range(N_CORES):
        sl = slice(c * B_SHARD, (c + 1) * B_SHARD)
        im = {"W": w}
        for m in range(NM):
            im[f"mod{m}"] = np.ascontiguousarray(
                np.asarray(inputs[f"mod{m}"], dtype=np.float32)[sl]
            )
        in_maps.append(im)
    return run_bass_kernel_spmd(
        nc, in_maps, core_ids=list(range(N_CORES)), trace=trace
    )


def kernel(**inputs) -> np.ndarray:
    res = _run(inputs, trace=False)
    return np.concatenate(
        [res.results[c]["out"] for c in range(N_CORES)], axis=0
    ).astype(np.float32)
